# revision 14
# baseline (speedup 1.0000x reference)
"""Trainium2 Bass kernel for HeadTailBoundaryPredictor.

Reference computation (B=8, S=512, E=16, H=768):
    t   = token @ Wt.T + bt                    [B,S,H]
    e2  = ent @ We.T + be                      [B,E,H]
    cls = einsum('besh,h->bes', relu(t[:,None]+e2[:,:,None]), wb)
    cls = where(mask, cls, -1e4); p = sigmoid(cls)

Math restructure: fold wb into the projections. With a = |wb|, s = sign(wb):
    cls[e,s] = sum_o s[o] * relu( a[o]*t[s,o] + a[o]*e2[e,o] )
since a[o]*relu(x) = relu(a[o]*x) for a >= 0.

Device plan (per core = one batch, data-parallel over B):
  - Host compacts the sequence dim: only token positions with mask=1 are
    shipped/computed (S_c = roundup(max_count, 64)); masked outputs are the
    constants -1e4 / sigmoid(-1e4)=0, filled host-side.
  - token/Wt/We/ent are bf16 (halves DMA); u/acts are f16 so the DVE runs
    tensor_scalar in 4x mode; reduce matmuls are f16 (full PE rate).
  - u'T[o,s]  = (diag(a) Wt tokenT)   per o-chunk j, bf16 matmuls (TensorE)
  - v''T[o,e] = (diag(a) We entT) + a*(bt+be)   (TensorE + bias add)
  - act[o,s]  = relu(u' + v''[:,e]) f16, per entity: 13 on VectorE (4x mode),
    2 on ScalarE, 1 on GpSimd
  - cls[e,s]  = sgnT @ act   via 1-column f16 matmuls rotated over the 4
    PE column groups (tile_position) so up to 4 run concurrently
  - p = sigmoid(cls) (ScalarE); outputs f16, host casts/scatters.
  - All DRAM inputs are host-packed partition-major so each is a single
    contiguous-per-partition DMA; descriptor gen is split across the
    SP and ACT hardware DGE queues (weights j-sliced to unblock compute).
"""

import sys

for _p in ("/opt/trn_rl_repo", "/root/.axon_site/_ro/trn_rl_repo"):
    if _p not in sys.path:
        sys.path.append(_p)

import numpy as np
import ml_dtypes

import concourse.bass as bass
import concourse.mybir as mybir
import concourse.tile as tile
from concourse.bass_utils import run_bass_kernel_spmd

dt = mybir.dt
AF = mybir.ActivationFunctionType
ALU = mybir.AluOpType

B, S, E, H = 8, 512, 16, 768
P = 128
NH = H // P  # 6 chunks of the hidden/output dims
NQ = E // 4  # 4 entity quads (one PSUM bank each)

N_WARMUP = 4
N_DVE = 11  # entities 0..10 on VectorE (from u_sb f16); 11..15 on ScalarE (from PSUM)
# reduce consumption order: ScalarE entities are ready first (no u-copy dep),
# interleaved so consecutive matmuls rotate PE column groups (e % 4)
REDUCE_ORDER = [11, 12, 13, 14, 15, 0, 1, 2, 3, 4, 5, 6, 7, 8, 9, 10]

_WAITSPLIT_CTR = [0]


def _split_excess_waits(nc, limit=1):
    """walrus (CoreV3) accepts at most `limit` sync-wait commands per
    instruction; Tile can emit more (e.g. the tail drain). Move excess waits
    onto freshly inserted same-engine NoOps, which is semantically identical."""
    n = 0
    for f in nc.m.functions:
        for bb in f.blocks:
            insts = list(bb.instructions)
            out = []
            changed = False
            for inst in insts:
                si = inst.sync_info
                waits = list(si.on_wait) if si else []
                if len(waits) > limit:
                    head, tail = waits[:-limit], waits[-limit:]
                    for i in range(0, len(head), limit):
                        _WAITSPLIT_CTR[0] += 1
                        nop = mybir.InstNoOp(
                            name=f"waitsplit_nop_{_WAITSPLIT_CTR[0]}", ins=[], outs=[]
                        )
                        nop.engine = inst.engine
                        nop.sync_info = mybir.SyncInfo(
                            on_wait=head[i : i + limit], on_update=[]
                        )
                        out.append(nop)
                        n += 1
                    si.on_wait = tail
                    inst.sync_info = si
                    changed = True
                out.append(inst)
            if changed:
                bb.instructions = out
    return n


def _build_nc(S_c):
    nc = bass.Bass()

    tok_pk = nc.dram_tensor("tok_pk", [P, NH * S_c], dt.bfloat16, kind="ExternalInput")
    wt_pk = nc.dram_tensor("wt_pk", [P, NH * NH * P], dt.bfloat16, kind="ExternalInput")
    we_pk = nc.dram_tensor("we_pk", [P, NH * NH * P], dt.bfloat16, kind="ExternalInput")
    ent_pk = nc.dram_tensor("ent_pk", [P, NH * E], dt.bfloat16, kind="ExternalInput")
    sgn_pk = nc.dram_tensor("sgn_pk", [P, NH], dt.float16, kind="ExternalInput")
    bb_pk = nc.dram_tensor("bb_pk", [P, NH], dt.float32, kind="ExternalInput")

    # out[q, g, c]: entity e = 4q+g; c = [cls | p] each S_c wide
    out_t = nc.dram_tensor("out", [NQ, 4, 2 * S_c], dt.float16, kind="ExternalOutput")

    with tile.TileContext(nc) as tc:
        with (
            tc.tile_pool(name="const", bufs=1) as cpool,
            tc.tile_pool(name="wts", bufs=1) as wpool,
            tc.tile_pool(name="usb", bufs=NH) as upool,
            tc.tile_pool(name="acts", bufs=48) as apool,
            tc.tile_pool(name="outs", bufs=1) as opool,
            tc.tile_pool(name="psv", bufs=1, space="PSUM") as psv,
            tc.tile_pool(name="psu", bufs=3, space="PSUM") as psu,
            tc.tile_pool(name="psc", bufs=1, space="PSUM") as psc,
        ):
            # ---- SBUF tiles ----
            t_sgn = cpool.tile([P, NH], dt.float16, tag="sgn")
            t_bb = cpool.tile([P, NH], dt.float32, tag="bb")
            t_dmy = cpool.tile([P, 1], dt.float32, tag="dmy")
            v_sb = cpool.tile([P, NH * E], dt.float32, tag="vsb")
            t_wt = wpool.tile([P, NH * NH * P], dt.bfloat16, tag="wt")
            t_we = wpool.tile([P, NH * NH * P], dt.bfloat16, tag="we")
            t_tok = wpool.tile([P, NH * S_c], dt.bfloat16, tag="tok")
            t_ent = wpool.tile([P, NH * E], dt.bfloat16, tag="ent")
            osb = opool.tile([P, NQ * 2 * S_c], dt.float16, tag="osb")

            # ---- DMA issue. ACT queue: consts + We (j-sliced) ----
            nc.scalar.dma_start(t_sgn[:], sgn_pk[:])
            nc.scalar.dma_start(t_bb[:], bb_pk[:])
            for j in range(3):
                sl = slice(j * NH * P, (j + 1) * NH * P)
                nc.scalar.dma_start(t_we[:, sl], we_pk[:, sl])
            # SP queue: token + Wt (j-sliced)
            nc.sync.dma_start(t_tok[:], tok_pk[:])
            for j in range(NH):
                sl = slice(j * NH * P, (j + 1) * NH * P)
                nc.sync.dma_start(t_wt[:, sl], wt_pk[:, sl])
            # GpSimd queue (SWDGE): entities, tail of We
            nc.gpsimd.dma_start(t_ent[:], ent_pk[:])
            for j in range(3, NH):
                sl = slice(j * NH * P, (j + 1) * NH * P)
                nc.gpsimd.dma_start(t_we[:, sl], we_pk[:, sl])

            # dummy sigmoid pulls the ACT table load off the critical path
            nc.scalar.activation(t_dmy[:], t_bb[:, 0:1], AF.Sigmoid)

            # ---- main pipeline (no warmup: it starts too late to ramp the
            # p-state and only delays the first real matmul) ----
            ps_v = psv.tile([P, NH * E], dt.float32, tag="vps")
            u_sb = [None] * NH
            act_t = [[None] * E for _ in range(NH)]
            ps_c = [
                psc.tile([P, S_c], dt.float32, tag=f"cq{q}", name=f"ps_c{q}")
                for q in range(NQ)
            ]

            def vproj(j):
                # v''T[o in chunk j, e] accumulated over k; bias added on copy
                for k in range(NH):
                    nc.tensor.matmul(
                        ps_v[:, j * E : (j + 1) * E],
                        t_we[:, (j * NH + k) * P : (j * NH + k + 1) * P],
                        t_ent[:, k * E : (k + 1) * E],
                        start=(k == 0),
                        stop=(k == NH - 1),
                    )
                nc.vector.tensor_scalar(
                    v_sb[:, j * E : (j + 1) * E],
                    ps_v[:, j * E : (j + 1) * E],
                    t_bb[:, j : j + 1],
                    None,
                    op0=ALU.add,
                )

            ps_u_t = [None] * NH

            def uproj(j):
                ps_u = psu.tile([P, S_c], dt.float32, tag="ups", name=f"ps_u{j}")
                ps_u_t[j] = ps_u
                for k in range(NH):
                    nc.tensor.matmul(
                        ps_u[:],
                        t_wt[:, (j * NH + k) * P : (j * NH + k + 1) * P],
                        t_tok[:, k * S_c : (k + 1) * S_c],
                        start=(k == 0),
                        stop=(k == NH - 1),
                    )
                u_sb[j] = upool.tile([P, S_c], dt.float16, tag="u", name=f"u_sb{j}")
                nc.scalar.copy(u_sb[j][:], ps_u[:])

            def acts(j):
                # ScalarE entities first (read PSUM directly; no u-copy dep)
                for e in range(N_DVE, E):
                    a = apool.tile([P, S_c], dt.float16, tag="act", name=f"act_{j}_{e}")
                    act_t[j][e] = a
                    bias = v_sb[:, j * E + e : j * E + e + 1]
                    nc.scalar.activation(a[:], ps_u_t[j][:], AF.Relu, bias=bias)
                for e in range(N_DVE):
                    a = apool.tile([P, S_c], dt.float16, tag="act", name=f"act_{j}_{e}")
                    act_t[j][e] = a
                    bias = v_sb[:, j * E + e : j * E + e + 1]
                    nc.vector.tensor_scalar(
                        a[:], u_sb[j][:], bias, 0.0, op0=ALU.add, op1=ALU.max
                    )

            def reduce(j):
                for e in REDUCE_ORDER:
                    q, g = e // 4, e % 4
                    nc.tensor.matmul(
                        ps_c[q][32 * g : 32 * g + 1, :],
                        t_sgn[:, j : j + 1],
                        act_t[j][e][:],
                        start=(j == 0),
                        stop=(j == NH - 1),
                        tile_position=(0, 32 * g),
                    )

            # PE program order: warmup, then per-j u/v interleaved with
            # reduce lagging one chunk. u-proj first: its DMA deps (tok+wt)
            # land before v-proj's (we+ent), so the PE starts sooner.
            uproj(0)
            vproj(0)
            acts(0)
            uproj(1)
            vproj(1)
            acts(1)
            reduce(0)
            for j in range(2, NH):
                uproj(j)
                vproj(j)
                acts(j)
                reduce(j - 1)

            # last chunk: finish one quad at a time and start its tail
            # (cls copy on DVE, sigmoid on ACT) while later quads reduce.
            j = NH - 1
            for q in range(NQ):
                for g in range(4):
                    e = 4 * q + g
                    nc.tensor.matmul(
                        ps_c[q][32 * g : 32 * g + 1, :],
                        t_sgn[:, j : j + 1],
                        act_t[j][e][:],
                        start=False,
                        stop=True,
                        tile_position=(0, 32 * g),
                    )
                nc.vector.tensor_scalar(
                    osb[:, q * 2 * S_c : q * 2 * S_c + S_c],
                    ps_c[q][:],
                    0.0,
                    None,
                    op0=ALU.add,
                )
                nc.scalar.activation(
                    osb[:, q * 2 * S_c + S_c : (q + 1) * 2 * S_c],
                    ps_c[q][:],
                    AF.Sigmoid,
                )
            try:
                src = osb[0 : P : 32, :].rearrange("p (q c) -> p q c", q=NQ)
                dst = out_t[:].rearrange("q g c -> g q c")
                nc.sync.dma_start(dst, src)
            except Exception:
                for q in range(NQ):
                    nc.sync.dma_start(
                        out_t[q, :, :],
                        osb[0 : P : 32, q * 2 * S_c : (q + 1) * 2 * S_c],
                    )

    _split_excess_waits(nc, limit=1)
    return nc


_NC_CACHE = {}


def _get_nc(S_c):
    if S_c not in _NC_CACHE:
        _NC_CACHE[S_c] = _build_nc(S_c)
    return _NC_CACHE[S_c]


def _pack_pmajor(mat, ncols):
    """[H, ncols] -> [P, NH*ncols] partition-major: out[p, k*ncols+c] =
    mat[k*P+p, c]."""
    return np.ascontiguousarray(
        mat.reshape(NH, P, ncols).transpose(1, 0, 2).reshape(P, NH * ncols)
    )


def kernel(token_embedding, entity_embedding, token_mask, Wt, bt, We, be, wb, **kw):
    token_embedding = np.asarray(token_embedding, dtype=np.float32)
    entity_embedding = np.asarray(entity_embedding, dtype=np.float32)
    token_mask = np.asarray(token_mask).astype(bool)
    Wt = np.asarray(Wt, dtype=np.float32)
    bt = np.asarray(bt, dtype=np.float32)
    We = np.asarray(We, dtype=np.float32)
    be = np.asarray(be, dtype=np.float32)
    wb = np.asarray(wb, dtype=np.float32)

    bf16 = ml_dtypes.bfloat16

    a = np.abs(wb)
    sgn = np.where(wb >= 0, np.float32(1.0), np.float32(-1.0))

    # fold |wb| into the weights; transpose to [h, o]
    W2t = (Wt * a[:, None]).T.astype(np.float32)  # [h, o]
    W2e = (We * a[:, None]).T.astype(np.float32)
    bb = ((bt + be) * a).astype(np.float32)

    # wt_pk[p, (j*NH+k)*P + c] = W2[k*P+p, j*P+c]  (j-major blocks)
    def pack_w(W2):
        arr = W2.reshape(NH, P, NH, P).transpose(1, 2, 0, 3)  # [p, j, k, c]
        return np.ascontiguousarray(arr.reshape(P, NH * NH * P)).astype(bf16)

    wt_pk = pack_w(W2t)
    we_pk = pack_w(W2e)
    sgn_pk = np.ascontiguousarray(sgn.reshape(NH, P).T).astype(np.float16)
    bb_pk = np.ascontiguousarray(bb.reshape(NH, P).T).astype(np.float32)

    idxs = [np.nonzero(token_mask[b])[0] for b in range(B)]
    nmax = max((len(ix) for ix in idxs), default=1)
    S_c = max(64, -(-nmax // 32) * 32)

    nc = _get_nc(S_c)
    in_maps = []
    for b in range(B):
        ix = idxs[b]
        tokc = np.zeros((S_c, H), dtype=np.float32)
        tokc[: len(ix)] = token_embedding[b][ix]
        tok_pk = _pack_pmajor(tokc.T, S_c).astype(bf16)  # [P, NH*S_c]
        ent_pk = _pack_pmajor(entity_embedding[b].T, E).astype(bf16)
        in_maps.append(
            {
                "tok_pk": tok_pk,
                "wt_pk": wt_pk,
                "we_pk": we_pk,
                "ent_pk": ent_pk,
                "sgn_pk": sgn_pk,
                "bb_pk": bb_pk,
            }
        )

    res = run_bass_kernel_spmd(nc, in_maps, core_ids=list(range(B)))

    cls = np.full((B, E, S), -10000.0, dtype=np.float32)
    p = np.zeros((B, E, S), dtype=np.float32)
    for b in range(B):
        o = np.asarray(res.results[b]["out"], dtype=np.float32).reshape(E, 2 * S_c)
        ix = idxs[b]
        cls[b][:, ix] = o[:, : len(ix)]
        p[b][:, ix] = o[:, S_c : S_c + len(ix)]
    return cls, p


# revision 18
# speedup vs baseline: 1.0323x; 1.0323x over previous
"""Trainium2 Bass kernel for HeadTailBoundaryPredictor.

Reference computation (B=8, S=512, E=16, H=768):
    t   = token @ Wt.T + bt                    [B,S,H]
    e2  = ent @ We.T + be                      [B,E,H]
    cls = einsum('besh,h->bes', relu(t[:,None]+e2[:,:,None]), wb)
    cls = where(mask, cls, -1e4); p = sigmoid(cls)

Math restructure: fold wb into the projections. With a = |wb|, s = sign(wb):
    cls[e,s] = sum_o s[o] * relu( a[o]*t[s,o] + a[o]*e2[e,o] )
since a[o]*relu(x) = relu(a[o]*x) for a >= 0.

Device plan (per core = one batch, data-parallel over B):
  - Host compacts the sequence dim: only token positions with mask=1 are
    shipped/computed (S_c = roundup(max_count, 64)); masked outputs are the
    constants -1e4 / sigmoid(-1e4)=0, filled host-side.
  - token/Wt/We/ent are bf16 (halves DMA); u/acts are f16 so the DVE runs
    tensor_scalar in 4x mode; reduce matmuls are f16 (full PE rate).
  - u'T[o,s]  = (diag(a) Wt tokenT)   per o-chunk j, bf16 matmuls (TensorE)
  - v''T[o,e] = (diag(a) We entT) + a*(bt+be)   (TensorE + bias add)
  - act[o,s]  = relu(u' + v''[:,e]) f16, per entity: 13 on VectorE (4x mode),
    2 on ScalarE, 1 on GpSimd
  - cls[e,s]  = sgnT @ act   via 1-column f16 matmuls rotated over the 4
    PE column groups (tile_position) so up to 4 run concurrently
  - p = sigmoid(cls) (ScalarE); outputs f16, host casts/scatters.
  - All DRAM inputs are host-packed partition-major so each is a single
    contiguous-per-partition DMA; descriptor gen is split across the
    SP and ACT hardware DGE queues (weights j-sliced to unblock compute).
"""

import sys

for _p in ("/opt/trn_rl_repo", "/root/.axon_site/_ro/trn_rl_repo"):
    if _p not in sys.path:
        sys.path.append(_p)

import numpy as np
import ml_dtypes

import concourse.bass as bass
import concourse.mybir as mybir
import concourse.tile as tile
from concourse.bass_utils import run_bass_kernel_spmd

dt = mybir.dt
AF = mybir.ActivationFunctionType
ALU = mybir.AluOpType

B, S, E, H = 8, 512, 16, 768
P = 128
NH = H // P  # 6 chunks of the hidden/output dims
NQ = E // 4  # 4 entity quads (one PSUM bank each)

N_WARMUP = 4
N_DVE = 11  # entities 0..10 on VectorE (from u_sb f16); 11..15 on ScalarE (from PSUM)
# reduce consumption order: ScalarE entities are ready first (no u-copy dep),
# interleaved so consecutive matmuls rotate PE column groups (e % 4)
REDUCE_ORDER = [11, 12, 13, 14, 15, 0, 1, 2, 3, 4, 5, 6, 7, 8, 9, 10]

_WAITSPLIT_CTR = [0]


def _split_excess_waits(nc, limit=1):
    """walrus (CoreV3) accepts at most `limit` sync-wait commands per
    instruction; Tile can emit more (e.g. the tail drain). Move excess waits
    onto freshly inserted same-engine NoOps, which is semantically identical."""
    n = 0
    for f in nc.m.functions:
        for bb in f.blocks:
            insts = list(bb.instructions)
            out = []
            changed = False
            for inst in insts:
                si = inst.sync_info
                waits = list(si.on_wait) if si else []
                if len(waits) > limit:
                    head, tail = waits[:-limit], waits[-limit:]
                    for i in range(0, len(head), limit):
                        _WAITSPLIT_CTR[0] += 1
                        nop = mybir.InstNoOp(
                            name=f"waitsplit_nop_{_WAITSPLIT_CTR[0]}", ins=[], outs=[]
                        )
                        nop.engine = inst.engine
                        nop.sync_info = mybir.SyncInfo(
                            on_wait=head[i : i + limit], on_update=[]
                        )
                        out.append(nop)
                        n += 1
                    si.on_wait = tail
                    inst.sync_info = si
                    changed = True
                out.append(inst)
            if changed:
                bb.instructions = out
    return n


def _build_nc(S_c):
    nc = bass.Bass()

    tok_pk = nc.dram_tensor("tok_pk", [P, NH * S_c], dt.bfloat16, kind="ExternalInput")
    wt_pk = nc.dram_tensor("wt_pk", [P, NH * NH * P], dt.bfloat16, kind="ExternalInput")
    we_pk = nc.dram_tensor("we_pk", [P, NH * NH * P], dt.bfloat16, kind="ExternalInput")
    ent_pk = nc.dram_tensor("ent_pk", [P, NH * E], dt.bfloat16, kind="ExternalInput")
    sgn_pk = nc.dram_tensor("sgn_pk", [P, NH], dt.float16, kind="ExternalInput")
    bb_pk = nc.dram_tensor("bb_pk", [P, NH], dt.float32, kind="ExternalInput")

    # out[q, g, c]: entity e = 4q+g; c = [cls | p] each S_c wide
    out_t = nc.dram_tensor("out", [NQ, 4, 2 * S_c], dt.float16, kind="ExternalOutput")

    with tile.TileContext(nc) as tc:
        with (
            tc.tile_pool(name="const", bufs=1) as cpool,
            tc.tile_pool(name="wts", bufs=1) as wpool,
            tc.tile_pool(name="usb", bufs=NH) as upool,
            tc.tile_pool(name="acts", bufs=48) as apool,
            tc.tile_pool(name="outs", bufs=1) as opool,
            tc.tile_pool(name="psw", bufs=1, space="PSUM") as psw,
            tc.tile_pool(name="psv", bufs=1, space="PSUM") as psv,
            tc.tile_pool(name="psu", bufs=2, space="PSUM") as psu,
            tc.tile_pool(name="psc", bufs=1, space="PSUM") as psc,
        ):
            # ---- SBUF tiles ----
            t_sgn = cpool.tile([P, NH], dt.float16, tag="sgn")
            t_bb = cpool.tile([P, NH], dt.float32, tag="bb")
            t_dmy = cpool.tile([P, 1], dt.float32, tag="dmy")
            t_wscr = cpool.tile([P, 256], dt.float16, tag="wscr")
            v_sb = cpool.tile([P, NH * E], dt.float32, tag="vsb")
            t_wt = wpool.tile([P, NH * NH * P], dt.bfloat16, tag="wt")
            t_we = wpool.tile([P, NH * NH * P], dt.bfloat16, tag="we")
            t_tok = wpool.tile([P, NH * S_c], dt.bfloat16, tag="tok")
            t_ent = wpool.tile([P, NH * E], dt.bfloat16, tag="ent")
            osb = opool.tile([P, NQ * 2 * S_c], dt.float16, tag="osb")

            # ---- DMA issue. ACT queue: consts + We (j-sliced) ----
            nc.scalar.dma_start(t_sgn[:], sgn_pk[:])
            nc.scalar.dma_start(t_bb[:], bb_pk[:])
            for j in range(3):
                sl = slice(j * NH * P, (j + 1) * NH * P)
                nc.scalar.dma_start(t_we[:, sl], we_pk[:, sl])
            # SP queue: token + Wt (j-sliced)
            nc.sync.dma_start(t_tok[:], tok_pk[:])
            for j in range(NH):
                sl = slice(j * NH * P, (j + 1) * NH * P)
                nc.sync.dma_start(t_wt[:, sl], wt_pk[:, sl])
            # GpSimd queue (SWDGE): scratch init, entities, tail of We
            nc.gpsimd.memset(t_wscr[:], 0.0)
            nc.gpsimd.dma_start(t_ent[:], ent_pk[:])
            for j in range(3, NH):
                sl = slice(j * NH * P, (j + 1) * NH * P)
                nc.gpsimd.dma_start(t_we[:, sl], we_pk[:, sl])

            # dummy sigmoid pulls the ACT table load off the critical path
            nc.scalar.activation(t_dmy[:], t_bb[:, 0:1], AF.Sigmoid)

            # ---- PE warmup (p-state ramp) on a scratch PSUM bank ----
            ps_w = psw.tile([P, 256], dt.float32, tag="wps")
            for w in range(N_WARMUP):
                nc.tensor.matmul(
                    ps_w[0:1, :], t_sgn[:, 0:1], t_wscr[:], start=True, stop=True
                )

            # ---- main pipeline ----
            ps_v = psv.tile([P, NH * E], dt.float32, tag="vps")
            u_sb = [None] * NH
            act_t = [[None] * E for _ in range(NH)]
            ps_c = [
                psc.tile([P, S_c], dt.float32, tag=f"cq{q}", name=f"ps_c{q}")
                for q in range(NQ)
            ]

            def vproj(j):
                # v''T[o in chunk j, e] accumulated over k; bias added on copy
                for k in range(NH):
                    nc.tensor.matmul(
                        ps_v[:, j * E : (j + 1) * E],
                        t_we[:, (j * NH + k) * P : (j * NH + k + 1) * P],
                        t_ent[:, k * E : (k + 1) * E],
                        start=(k == 0),
                        stop=(k == NH - 1),
                    )
                nc.vector.tensor_scalar(
                    v_sb[:, j * E : (j + 1) * E],
                    ps_v[:, j * E : (j + 1) * E],
                    t_bb[:, j : j + 1],
                    None,
                    op0=ALU.add,
                )

            ps_u_t = [None] * NH

            def uproj(j):
                ps_u = psu.tile([P, S_c], dt.float32, tag="ups", name=f"ps_u{j}")
                ps_u_t[j] = ps_u
                for k in range(NH):
                    nc.tensor.matmul(
                        ps_u[:],
                        t_wt[:, (j * NH + k) * P : (j * NH + k + 1) * P],
                        t_tok[:, k * S_c : (k + 1) * S_c],
                        start=(k == 0),
                        stop=(k == NH - 1),
                    )
                u_sb[j] = upool.tile([P, S_c], dt.float16, tag="u", name=f"u_sb{j}")
                nc.scalar.copy(u_sb[j][:], ps_u[:])

            def acts(j):
                # ScalarE entities first (read PSUM directly; no u-copy dep)
                for e in range(N_DVE, E):
                    a = apool.tile([P, S_c], dt.float16, tag="act", name=f"act_{j}_{e}")
                    act_t[j][e] = a
                    bias = v_sb[:, j * E + e : j * E + e + 1]
                    nc.scalar.activation(a[:], ps_u_t[j][:], AF.Relu, bias=bias)
                for e in range(N_DVE):
                    a = apool.tile([P, S_c], dt.float16, tag="act", name=f"act_{j}_{e}")
                    act_t[j][e] = a
                    bias = v_sb[:, j * E + e : j * E + e + 1]
                    nc.vector.tensor_scalar(
                        a[:], u_sb[j][:], bias, 0.0, op0=ALU.add, op1=ALU.max
                    )

            def reduce(j):
                for e in REDUCE_ORDER:
                    q, g = e // 4, e % 4
                    nc.tensor.matmul(
                        ps_c[q][32 * g : 32 * g + 1, :],
                        t_sgn[:, j : j + 1],
                        act_t[j][e][:],
                        start=(j == 0),
                        stop=(j == NH - 1),
                        tile_position=(0, 32 * g),
                    )

            # PE program order: warmup, then per-j u/v interleaved with
            # reduce lagging one chunk. u-proj first: its DMA deps (tok+wt)
            # land before v-proj's (we+ent), so the PE starts sooner.
            uproj(0)
            vproj(0)
            acts(0)
            uproj(1)
            vproj(1)
            acts(1)
            reduce(0)
            for j in range(2, NH):
                uproj(j)
                vproj(j)
                acts(j)
                reduce(j - 1)

            # last chunk: finish one quad at a time and start its tail
            # (cls copy on DVE, sigmoid on ACT) while later quads reduce.
            j = NH - 1
            for q in range(NQ):
                for g in range(4):
                    e = 4 * q + g
                    nc.tensor.matmul(
                        ps_c[q][32 * g : 32 * g + 1, :],
                        t_sgn[:, j : j + 1],
                        act_t[j][e][:],
                        start=False,
                        stop=True,
                        tile_position=(0, 32 * g),
                    )
                nc.vector.tensor_scalar(
                    osb[:, q * 2 * S_c : q * 2 * S_c + S_c],
                    ps_c[q][:],
                    0.0,
                    None,
                    op0=ALU.add,
                )
                nc.scalar.activation(
                    osb[:, q * 2 * S_c + S_c : (q + 1) * 2 * S_c],
                    ps_c[q][:],
                    AF.Sigmoid,
                )
            try:
                src = osb[0 : P : 32, :].rearrange("p (q c) -> p q c", q=NQ)
                dst = out_t[:].rearrange("q g c -> g q c")
                nc.sync.dma_start(dst, src)
            except Exception:
                for q in range(NQ):
                    nc.sync.dma_start(
                        out_t[q, :, :],
                        osb[0 : P : 32, q * 2 * S_c : (q + 1) * 2 * S_c],
                    )

    _split_excess_waits(nc, limit=1)
    return nc


_NC_CACHE = {}


def _get_nc(S_c):
    if S_c not in _NC_CACHE:
        _NC_CACHE[S_c] = _build_nc(S_c)
    return _NC_CACHE[S_c]


def _pack_pmajor(mat, ncols):
    """[H, ncols] -> [P, NH*ncols] partition-major: out[p, k*ncols+c] =
    mat[k*P+p, c]."""
    return np.ascontiguousarray(
        mat.reshape(NH, P, ncols).transpose(1, 0, 2).reshape(P, NH * ncols)
    )


def kernel(token_embedding, entity_embedding, token_mask, Wt, bt, We, be, wb, **kw):
    token_embedding = np.asarray(token_embedding, dtype=np.float32)
    entity_embedding = np.asarray(entity_embedding, dtype=np.float32)
    token_mask = np.asarray(token_mask).astype(bool)
    Wt = np.asarray(Wt, dtype=np.float32)
    bt = np.asarray(bt, dtype=np.float32)
    We = np.asarray(We, dtype=np.float32)
    be = np.asarray(be, dtype=np.float32)
    wb = np.asarray(wb, dtype=np.float32)

    bf16 = ml_dtypes.bfloat16

    a = np.abs(wb)
    sgn = np.where(wb >= 0, np.float32(1.0), np.float32(-1.0))

    # fold |wb| into the weights; transpose to [h, o]
    W2t = (Wt * a[:, None]).T.astype(np.float32)  # [h, o]
    W2e = (We * a[:, None]).T.astype(np.float32)
    bb = ((bt + be) * a).astype(np.float32)

    # wt_pk[p, (j*NH+k)*P + c] = W2[k*P+p, j*P+c]  (j-major blocks)
    def pack_w(W2):
        arr = W2.reshape(NH, P, NH, P).transpose(1, 2, 0, 3)  # [p, j, k, c]
        return np.ascontiguousarray(arr.reshape(P, NH * NH * P)).astype(bf16)

    wt_pk = pack_w(W2t)
    we_pk = pack_w(W2e)
    sgn_pk = np.ascontiguousarray(sgn.reshape(NH, P).T).astype(np.float16)
    bb_pk = np.ascontiguousarray(bb.reshape(NH, P).T).astype(np.float32)

    idxs = [np.nonzero(token_mask[b])[0] for b in range(B)]
    nmax = max((len(ix) for ix in idxs), default=1)
    S_c = max(64, -(-nmax // 32) * 32)

    nc = _get_nc(S_c)
    in_maps = []
    for b in range(B):
        ix = idxs[b]
        tokc = np.zeros((S_c, H), dtype=np.float32)
        tokc[: len(ix)] = token_embedding[b][ix]
        tok_pk = _pack_pmajor(tokc.T, S_c).astype(bf16)  # [P, NH*S_c]
        ent_pk = _pack_pmajor(entity_embedding[b].T, E).astype(bf16)
        in_maps.append(
            {
                "tok_pk": tok_pk,
                "wt_pk": wt_pk,
                "we_pk": we_pk,
                "ent_pk": ent_pk,
                "sgn_pk": sgn_pk,
                "bb_pk": bb_pk,
            }
        )

    res = run_bass_kernel_spmd(nc, in_maps, core_ids=list(range(B)))

    cls = np.full((B, E, S), -10000.0, dtype=np.float32)
    p = np.zeros((B, E, S), dtype=np.float32)
    for b in range(B):
        o = np.asarray(res.results[b]["out"], dtype=np.float32).reshape(E, 2 * S_c)
        ix = idxs[b]
        cls[b][:, ix] = o[:, : len(ix)]
        p[b][:, ix] = o[:, S_c : S_c + len(ix)]
    return cls, p
